# revision 38
# baseline (speedup 1.0000x reference)
"""Trainium2 Bass kernel for nn_DenselyCnnAttLayer.

Reference computation (B=64, S=512, L=6, D=512):
    X = stack([x0..x5], axis=2)                  # [B,S,L,D]
    s = X.sum(-1)                                # [B,S,L]
    logits = einsum('bsl,slm->bsm', s, Ws)       # [B,S,L]
    a = softmax(logits, -1)
    out = einsum('bsl,bsld->bsd', a, X)          # [B,S,D]

Strategy: data-parallel over batch across 8 cores (8 batches/core).
The kernel is HBM-bandwidth-bound, so inputs are cast to bf16 on the
host (tolerance is 2e-2; bf16 costs ~3e-3) and packed per 128-row block
as [block, P, J, D] so a group of blocks arrives as one large
fully-contiguous-per-partition DMA.  The output is stored as bf16
[block, P, D] and unpacked / upcast on the host.

Per 128-row block the compute is spread over every engine so each stays
under the DMA roofline:
  - row sums   s[p,j] = sum_d x_j[p,d]   -> split DVE / POOL / ACT
    (tensor_scalar / activation with accum_out, outputs trashed)
  - logits+exp (tiny, batched across the blocks of a group) -> DVE+ACT
  - weighted layer sum  out = sum_j e_j * x_j   -> TENSOR ENGINE:
    six diagonal matmuls diag(e_j) @ x_j accumulated in one PSUM bank.
    diag(e_j) is built by DVE as ident * e_j (tensor_scalar, [P,128]).
  - normalize: ACT/DVE copy PSUM->SBUF with scale = 1/sum_j e_j (bf16)
  - groups taper to 2/1/1 blocks at the end so the post-load pipeline
    drain is short; all DMA rides the SP ring in program order.
"""

import os
import sys

for _p in ("/opt/trn_rl_repo", "/root/.axon_site/_ro/trn_rl_repo"):
    if os.path.isdir(_p) and _p not in sys.path:
        sys.path.insert(0, _p)
        break

import numpy as np

import concourse.bass as bass
import concourse.bacc as bacc
import concourse.mybir as mybir
from concourse import tile
from concourse.bass_utils import run_bass_kernel_spmd

B, S, L, D = 64, 512, 6, 512
N_CORES = 8
B_PER = B // N_CORES       # 8 batches per core
ROWS = B_PER * S           # 4096 rows per core
P = 128                    # SBUF partitions
NB = ROWS // P             # 32 row blocks per core
JD = L * D                 # per-partition elements of one block load

FP32 = mybir.dt.float32
BF16 = mybir.dt.bfloat16
AF = mybir.ActivationFunctionType
ALU = mybir.AluOpType
AX = mybir.AxisListType

# Group plan: block-counts per pipeline group (loads, compute, stores all
# use this granularity).  Coarse in steady state (big DMAs), tapering at
# the end (short pipeline drain).
GROUPS = [1, 1, 2, 4, 4, 4, 4, 4, 4, 2, 1, 1]
assert sum(GROUPS) == NB
# groups must not straddle the S/P=4 ws-position boundary
_b = 0
for _kp in GROUPS:
    assert _b % 4 + _kp <= 4, (_b, _kp)
    _b += _kp

# Row-sum engine split per block (6 layer sums total).  DVE-heavy: a DVE
# bf16 tensor_scalar sum is ~4x cheaper than an ACT one.  The Pool engine
# cannot run tensor ops on real TRN2 (ISA opcode check), so sums go to
# DVE + ACT only.
N_DVE_SUMS = 4
N_ACT_SUMS = L - N_DVE_SUMS
# PSUM->SBUF normalize-copies per group handled by DVE (rest on ACT).
N_DVE_COPIES = 0


def build_module(reps: int = 1) -> bass.Bass:
    """Build the kernel module.  reps>1 unrolls the whole schedule reps
    times back-to-back (identical work, same outputs) — used only for
    steady-state hardware timing: (T(reps) - T(1)) / (reps - 1)."""
    nc = bacc.Bacc("TRN2", debug=False, num_devices=N_CORES)
    xall = nc.dram_tensor("xall", [NB * P, JD], BF16, kind="ExternalInput").ap()
    ws = nc.dram_tensor("ws", [P, 4 * L * L], FP32, kind="ExternalInput").ap()
    ident = nc.dram_tensor("ident", [P, P], BF16, kind="ExternalInput").ap()
    out = nc.dram_tensor("out", [NB * P, D], BF16, kind="ExternalOutput").ap()

    groups = []
    b0 = 0
    for kp in GROUPS:
        groups.append((b0, kp))
        b0 += kp

    with tile.TileContext(nc) as tc:
        with (
            tc.tile_pool(name="const", bufs=1) as cpool,
            tc.tile_pool(name="xpool", bufs=4) as xpool,
            tc.tile_pool(name="xtail", bufs=2) as xtail,
            tc.tile_pool(name="opool", bufs=5) as opool,
            tc.tile_pool(name="otail", bufs=1) as otail,
            tc.tile_pool(name="dpool", bufs=2) as dpool,
            tc.tile_pool(name="small", bufs=3) as small,
            tc.tile_pool(name="trash", bufs=1) as trashpool,
            tc.tile_pool(name="ps", bufs=7, space=bass.MemorySpace.PSUM) as pspool,
            tc.tile_pool(name="psd", bufs=1, space=bass.MemorySpace.PSUM) as psdpool,
        ):
            def load_group(gi, rep):
                b0, kp = groups[gi]
                pool = xpool if kp == 4 else xtail
                tag = f"xg{kp}"
                xt = pool.tile([P, kp, L, D], BF16, tag=tag,
                               name=f"xg_{rep}_{gi}")
                # split big loads in half so the first blocks' row sums can
                # start at the half-way point of the transfer; the tile is
                # k-major so each half is a contiguous SBUF range
                halves = [(0, kp)] if kp <= 2 else [(0, 2), (2, 4)]
                for k0, k1 in halves:
                    nc.sync.dma_start(
                        out=xt[:, k0:k1, :, :],
                        in_=xall[(b0 + k0) * P : (b0 + k1) * P, :].rearrange(
                            "(k p) (j d) -> p k j d", p=P, d=D
                        ),
                    )
                return xt

            # First group load goes out before the small constant loads
            # (which ride the ACT ring so they never delay a big load).
            first_x = load_group(0, 0)

            ws_t = cpool.tile([P, 4 * L * L], FP32, name="ws_t")
            nc.scalar.dma_start(out=ws_t[:, :], in_=ws[:, :])
            id_t = cpool.tile([P, P], BF16, name="id_t")
            nc.scalar.dma_start(out=id_t[:, :], in_=ident[:, :])

            # Garbage destinations for the row-sum accum trick (never
            # read).  One per engine so they never cross-sync on WAW.
            trash_act = trashpool.tile([P, D], BF16, name="trash_act")
            trash_dve = trashpool.tile([P, D], BF16, name="trash_dve")

            class Grp:
                __slots__ = ("xt", "e", "recip", "uid", "gi", "b0", "kp", "o_t", "dg")

            def stage_a(st: Grp):
                """Row sums + logits + exp for all blocks of a group."""
                uid, kp = st.uid, st.kp
                s_t = small.tile([P, kp * L], FP32, tag=f"s{kp}",
                                 name=f"s_{uid}")
                # Row sums via tensor_scalar/activation accum_out (outputs
                # trashed).  Scaled by 1/D; Ws is pre-scaled by D on the host
                # so logits are unchanged.
                # late groups: keep sums off the (sim-slow) ACT engine so
                # the post-load drain isn't serialized behind ACT's backlog
                n_dve = N_DVE_SUMS if st.gi < 8 else 5
                inv_d = 1.0 / D
                for j in range(L):
                    for k in range(kp):
                        acc = s_t[:, k * L + j : k * L + j + 1]
                        if j < n_dve:
                            nc.vector.tensor_scalar(
                                out=trash_dve[:, :], in0=st.xt[:, k, j, :],
                                scalar1=inv_d, scalar2=0.0,
                                op0=ALU.mult, op1=ALU.add,
                                accum_out=acc,
                            )
                        else:
                            nc.scalar.activation(
                                trash_act[:, :], st.xt[:, k, j, :], AF.Copy,
                                scale=inv_d,
                                accum_out=acc,
                            )
                # logits[p,k,m] = sum_l s[p,k,l] * Ws[pos(p,k),l,m]
                prod = small.tile([P, kp * L * L], FP32, tag=f"pr{kp}",
                                  name=f"pr_{uid}")
                wb = st.b0 % 4
                wslice = ws_t[:, wb * L * L : (wb + kp) * L * L]
                nc.vector.tensor_tensor(
                    out=prod[:, :].rearrange("p (k l m) -> p k l m", l=L, m=L),
                    in0=s_t[:, :].rearrange("p (k l) -> p k l", l=L)
                    .unsqueeze(3).broadcast_to((P, kp, L, L)),
                    in1=wslice.rearrange("p (k l m) -> p k l m", l=L, m=L),
                    op=ALU.mult,
                )
                lg = small.tile([P, kp * L], FP32, tag=f"lg{kp}",
                                name=f"lg_{uid}")
                nc.vector.tensor_reduce(
                    out=lg[:, :].rearrange("p (k m) -> p k m", m=L),
                    in_=prod[:, :].rearrange("p (k l m) -> p k m l", l=L, m=L),
                    axis=AX.X,
                    op=ALU.add,
                )
                # exp; |logits| < ~20 so fp32 exp is safe without max-sub
                st.e = small.tile([P, kp * L], FP32, tag=f"e{kp}",
                                  name=f"e_{uid}")
                nc.scalar.activation(st.e[:, :], lg[:, :], AF.Exp)
                se = small.tile([P, kp], FP32, tag=f"se{kp}", name=f"se_{uid}")
                nc.vector.tensor_reduce(
                    out=se[:, :],
                    in_=st.e[:, :].rearrange("p (k m) -> p k m", m=L),
                    axis=AX.X,
                    op=ALU.add,
                )
                st.recip = small.tile([P, kp], FP32, tag=f"rc{kp}",
                                      name=f"rc_{uid}")
                nc.vector.reciprocal(st.recip[:, :], se[:, :])

            def diag_build(st: Grp):
                """DVE: diag(e_j) tiles for every block of the group, in a
                single tensor_tensor op (ident and e broadcast against each
                other) — one instruction instead of kp*L tiny ones."""
                uid, kp = st.uid, st.kp
                dg = dpool.tile([P, kp * L * P], BF16, tag=f"dg{kp}",
                                name=f"dg_{uid}")
                st.dg = dg
                for k in range(kp):
                    for j in range(L):
                        nc.vector.tensor_scalar_mul(
                            dg[:, (k * L + j) * P : (k * L + j + 1) * P],
                            id_t[:, :],
                            st.e[:, k * L + j : k * L + j + 1],
                        )

            def mm_copy(st: Grp):
                """PE burst (kp*L diagonal matmuls) + normalize-copies."""
                uid, kp = st.uid, st.kp
                pool = opool if kp == 4 else otail
                tag = f"o{kp}"
                o_t = pool.tile([P, kp * D], BF16, tag=tag, name=f"o_{uid}")
                st.o_t = o_t
                dg = st.dg
                pss = []
                for k in range(kp):
                    ps = pspool.tile([P, D], FP32, tag="ps",
                                     name=f"ps_{uid}_{k}")
                    pss.append(ps)
                    for j in range(L):
                        nc.tensor.matmul(
                            ps[:, :],
                            dg[:, (k * L + j) * P : (k * L + j + 1) * P],
                            st.xt[:, k, j, :],
                            start=(j == 0),
                            stop=(j == L - 1),
                        )
                for k in range(kp):
                    if k < N_DVE_COPIES:
                        nc.vector.tensor_scalar_mul(
                            o_t[:, k * D : (k + 1) * D], pss[k][:, :],
                            st.recip[:, k : k + 1],
                        )
                    else:
                        nc.scalar.activation(
                            o_t[:, k * D : (k + 1) * D], pss[k][:, :], AF.Copy,
                            scale=st.recip[:, k : k + 1],
                        )
                    # keep-warm: a tiny matmul chained to each copy keeps the
                    # Tensor engine's busy streak alive through the idle
                    # window between group bursts, so the p-state model never
                    # demotes the next burst's clock.
                    dum = psdpool.tile([8, 8], FP32, tag="dum",
                                       name=f"dum_{uid}_{k}")
                    nc.tensor.matmul(
                        dum[:, :], id_t[:, 0:8],
                        o_t[:, k * D : k * D + 8],
                        start=True, stop=True,
                    )

            def store_group(st: Grp):
                nc.sync.dma_start(
                    out=out[st.b0 * P : (st.b0 + st.kp) * P, :].rearrange(
                        "(k p) d -> p k d", p=P
                    ),
                    in_=st.o_t[:, :].rearrange("p (k d) -> p k d", d=D),
                )

            # Software pipeline, 2-deep while loads remain:
            #   load(g+1) ; stage_a(g)+diags(g) ; matmuls+copies(g-1) ;
            #   store(g-2)
            # diags(g) are emitted at the end of stage_a(g), so when the
            # Tensor engine starts group g's burst (next iteration) every
            # diagonal is already resident — the burst never stalls and the
            # PE p-state ramps to full clock.
            NG = len(groups)
            hist: list[Grp] = []      # groups whose mm_copy is not yet emitted
            unstored: list[Grp] = []  # groups computed but not yet stored
            for rep in range(reps):
                for gi in range(NG):
                    uid = rep * NG + gi
                    st = Grp()
                    st.uid = uid
                    st.gi = gi
                    st.b0, st.kp = groups[gi]
                    st.xt = first_x if uid == 0 else cur_x  # noqa: F821
                    last = uid + 1 >= reps * NG
                    if not last:
                        cur_x = load_group((uid + 1) % NG, (uid + 1) // NG)
                        stage_a(st)
                        diag_build(st)
                        if hist:
                            done = hist.pop(0)
                            mm_copy(done)
                            unstored.append(done)
                        # store with a 2-group lag behind mm_copy so the
                        # trigger's o_t dependency is always satisfied
                        while len(unstored) > 3:
                            store_group(unstored.pop(0))
                        hist.append(st)
                    else:
                        # all loads issued: drain
                        stage_a(st)
                        diag_build(st)
                        while hist:
                            done = hist.pop(0)
                            mm_copy(done)
                            unstored.append(done)
                        while len(unstored) > 1:
                            store_group(unstored.pop(0))
                        mm_copy(st)
                        unstored.append(st)
                        while unstored:
                            store_group(unstored.pop(0))

    # Legalize for TRN2 (≤1 sync wait per instruction) + register alloc.
    nc.compile()
    return nc


_MODULE_CACHE: bass.Bass | None = None


def _get_module() -> bass.Bass:
    global _MODULE_CACHE
    if _MODULE_CACHE is None:
        _MODULE_CACHE = build_module()
    return _MODULE_CACHE


def make_in_maps(inputs: dict) -> list:
    bf16 = mybir.dt.np(BF16)
    # Ws[s, l, m] -> ws[p, (kb l m)] with s = kb*128 + p
    ws = (
        np.asarray(inputs["Ws"], dtype=np.float32)
        .reshape(4, P, L * L)
        .transpose(1, 0, 2)
        .reshape(P, 4 * L * L)
    ) * float(D)
    ws = np.ascontiguousarray(ws)
    ident = np.eye(P, dtype=bf16)
    # Per core, row r = bi*512 + s;  block b = r//128, p = r%128.
    # Device layout: xall[b*P + p, j*D + d] = x_j[bi, s, d]
    xs = np.stack(
        [np.asarray(inputs[f"x{j}"]).astype(bf16) for j in range(L)], axis=0
    )  # [J, B, S, D]
    in_maps = []
    for c in range(N_CORES):
        xc = xs[:, c * B_PER : (c + 1) * B_PER]          # [J, B_PER, S, D]
        xc = xc.reshape(L, NB, P, D).transpose(1, 2, 0, 3)
        m = {
            "xall": np.ascontiguousarray(xc).reshape(NB * P, JD),
            "ws": ws,
            "ident": ident,
        }
        in_maps.append(m)
    return in_maps


def kernel(**inputs) -> np.ndarray:
    nc = _get_module()
    in_maps = make_in_maps(inputs)
    res = run_bass_kernel_spmd(nc, in_maps, core_ids=list(range(N_CORES)))
    outs = []
    for c in range(N_CORES):
        oc = np.asarray(res.results[c]["out"])            # [NB*P, D] bf16
        oc = oc.reshape(NB, P, D).reshape(B_PER, S, D)
        outs.append(oc.astype(np.float32))
    return np.concatenate(outs, axis=0)


# revision 39
# speedup vs baseline: 1.4272x; 1.4272x over previous
"""Trainium2 Bass kernel for nn_DenselyCnnAttLayer.

Reference computation (B=64, S=512, L=6, D=512):
    X = stack([x0..x5], axis=2)                  # [B,S,L,D]
    s = X.sum(-1)                                # [B,S,L]
    logits = einsum('bsl,slm->bsm', s, Ws)       # [B,S,L]
    a = softmax(logits, -1)
    out = einsum('bsl,bsld->bsd', a, X)          # [B,S,D]

Strategy: data-parallel over batch across 8 cores (8 batches/core).
The kernel is HBM-bandwidth-bound, so inputs are cast to bf16 on the
host (tolerance is 2e-2; bf16 costs ~3e-3) and packed per 128-row block
as [block, P, J, D] so a group of blocks arrives as one large
fully-contiguous-per-partition DMA.  The output is stored as bf16
[block, P, D] and unpacked / upcast on the host.

Per 128-row block the compute is spread over every engine so each stays
under the DMA roofline:
  - row sums   s[p,j] = sum_d x_j[p,d]   -> split DVE / POOL / ACT
    (tensor_scalar / activation with accum_out, outputs trashed)
  - logits+exp (tiny, batched across the blocks of a group) -> DVE+ACT
  - weighted layer sum  out = sum_j e_j * x_j   -> TENSOR ENGINE:
    six diagonal matmuls diag(e_j) @ x_j accumulated in one PSUM bank.
    diag(e_j) is built by DVE as ident * e_j (tensor_scalar, [P,128]).
  - normalize: ACT/DVE copy PSUM->SBUF with scale = 1/sum_j e_j (bf16)
  - groups taper to 2/1/1 blocks at the end so the post-load pipeline
    drain is short; all DMA rides the SP ring in program order.
"""

import os
import sys

for _p in ("/opt/trn_rl_repo", "/root/.axon_site/_ro/trn_rl_repo"):
    if os.path.isdir(_p) and _p not in sys.path:
        sys.path.insert(0, _p)
        break

import numpy as np

import concourse.bass as bass
import concourse.bacc as bacc
import concourse.mybir as mybir
from concourse import tile
from concourse.bass_utils import run_bass_kernel_spmd

B, S, L, D = 64, 512, 6, 512
N_CORES = 8
B_PER = B // N_CORES       # 8 batches per core
ROWS = B_PER * S           # 4096 rows per core
P = 128                    # SBUF partitions
NB = ROWS // P             # 32 row blocks per core
JD = L * D                 # per-partition elements of one block load

FP32 = mybir.dt.float32
BF16 = mybir.dt.bfloat16
AF = mybir.ActivationFunctionType
ALU = mybir.AluOpType
AX = mybir.AxisListType

# Group plan: block-counts per pipeline group (loads, compute, stores all
# use this granularity).  Coarse in steady state (big DMAs), tapering at
# the end (short pipeline drain).
GROUPS = [1, 1, 2, 4, 4, 4, 4, 4, 4, 2, 1, 1]
assert sum(GROUPS) == NB
# groups must not straddle the S/P=4 ws-position boundary
_b = 0
for _kp in GROUPS:
    assert _b % 4 + _kp <= 4, (_b, _kp)
    _b += _kp

# Row-sum engine split per block (6 layer sums total).  DVE-heavy: a DVE
# bf16 tensor_scalar sum is ~4x cheaper than an ACT one.  The Pool engine
# cannot run tensor ops on real TRN2 (ISA opcode check), so sums go to
# DVE + ACT only.
N_DVE_SUMS = 2
N_ACT_SUMS = L - N_DVE_SUMS
# PSUM->SBUF normalize-copies per group handled by DVE (rest on ACT).
N_DVE_COPIES = 0


def build_module(reps: int = 1) -> bass.Bass:
    """Build the kernel module.  reps>1 unrolls the whole schedule reps
    times back-to-back (identical work, same outputs) — used only for
    steady-state hardware timing: (T(reps) - T(1)) / (reps - 1)."""
    nc = bacc.Bacc("TRN2", debug=False, num_devices=N_CORES)
    xall = nc.dram_tensor("xall", [NB * P, JD], BF16, kind="ExternalInput").ap()
    ws = nc.dram_tensor("ws", [P, 4 * L * L], FP32, kind="ExternalInput").ap()
    ident = nc.dram_tensor("ident", [P, P], BF16, kind="ExternalInput").ap()
    out = nc.dram_tensor("out", [NB * P, D], BF16, kind="ExternalOutput").ap()

    groups = []
    b0 = 0
    for kp in GROUPS:
        groups.append((b0, kp))
        b0 += kp

    with tile.TileContext(nc) as tc:
        with (
            tc.tile_pool(name="const", bufs=1) as cpool,
            tc.tile_pool(name="xpool", bufs=4) as xpool,
            tc.tile_pool(name="xtail", bufs=2) as xtail,
            tc.tile_pool(name="opool", bufs=5) as opool,
            tc.tile_pool(name="otail", bufs=1) as otail,
            tc.tile_pool(name="dpool", bufs=2) as dpool,
            tc.tile_pool(name="small", bufs=3) as small,
            tc.tile_pool(name="trash", bufs=1) as trashpool,
            tc.tile_pool(name="ps", bufs=7, space=bass.MemorySpace.PSUM) as pspool,
            tc.tile_pool(name="psd", bufs=1, space=bass.MemorySpace.PSUM) as psdpool,
        ):
            def load_group(gi, rep):
                b0, kp = groups[gi]
                pool = xpool if kp == 4 else xtail
                tag = f"xg{kp}"
                xt = pool.tile([P, kp, L, D], BF16, tag=tag,
                               name=f"xg_{rep}_{gi}")
                # split big loads in half so the first blocks' row sums can
                # start at the half-way point of the transfer; the tile is
                # k-major so each half is a contiguous SBUF range
                halves = [(0, kp)] if kp <= 2 else [(0, 2), (2, 4)]
                for k0, k1 in halves:
                    nc.sync.dma_start(
                        out=xt[:, k0:k1, :, :],
                        in_=xall[(b0 + k0) * P : (b0 + k1) * P, :].rearrange(
                            "(k p) (j d) -> p k j d", p=P, d=D
                        ),
                    )
                return xt

            # First group load goes out before the small constant loads
            # (which ride the ACT ring so they never delay a big load).
            first_x = load_group(0, 0)

            ws_t = cpool.tile([P, 4 * L * L], FP32, name="ws_t")
            nc.scalar.dma_start(out=ws_t[:, :], in_=ws[:, :])
            id_t = cpool.tile([P, P], BF16, name="id_t")
            nc.scalar.dma_start(out=id_t[:, :], in_=ident[:, :])

            # Garbage destinations for the row-sum accum trick (never
            # read).  One per engine so they never cross-sync on WAW.
            trash_act = trashpool.tile([P, D], BF16, name="trash_act")
            trash_dve = trashpool.tile([P, D], BF16, name="trash_dve")

            class Grp:
                __slots__ = ("xt", "e", "recip", "uid", "gi", "b0", "kp", "o_t", "dg")

            def stage_a(st: Grp):
                """Row sums + logits + exp for all blocks of a group."""
                uid, kp = st.uid, st.kp
                s_t = small.tile([P, kp * L], FP32, tag=f"s{kp}",
                                 name=f"s_{uid}")
                # Row sums via tensor_scalar/activation accum_out (outputs
                # trashed).  Scaled by 1/D; Ws is pre-scaled by D on the host
                # so logits are unchanged.
                # late groups: keep sums off the (sim-slow) ACT engine so
                # the post-load drain isn't serialized behind ACT's backlog
                n_dve = N_DVE_SUMS if st.gi < 8 else 3
                inv_d = 1.0 / D
                for j in range(L):
                    for k in range(kp):
                        acc = s_t[:, k * L + j : k * L + j + 1]
                        if j < n_dve:
                            nc.vector.tensor_scalar(
                                out=trash_dve[:, :], in0=st.xt[:, k, j, :],
                                scalar1=inv_d, scalar2=0.0,
                                op0=ALU.mult, op1=ALU.add,
                                accum_out=acc,
                            )
                        else:
                            nc.scalar.activation(
                                trash_act[:, :], st.xt[:, k, j, :], AF.Copy,
                                scale=inv_d,
                                accum_out=acc,
                            )
                # logits[p,k,m] = sum_l s[p,k,l] * Ws[pos(p,k),l,m]
                prod = small.tile([P, kp * L * L], FP32, tag=f"pr{kp}",
                                  name=f"pr_{uid}")
                wb = st.b0 % 4
                wslice = ws_t[:, wb * L * L : (wb + kp) * L * L]
                nc.vector.tensor_tensor(
                    out=prod[:, :].rearrange("p (k l m) -> p k l m", l=L, m=L),
                    in0=s_t[:, :].rearrange("p (k l) -> p k l", l=L)
                    .unsqueeze(3).broadcast_to((P, kp, L, L)),
                    in1=wslice.rearrange("p (k l m) -> p k l m", l=L, m=L),
                    op=ALU.mult,
                )
                lg = small.tile([P, kp * L], FP32, tag=f"lg{kp}",
                                name=f"lg_{uid}")
                nc.vector.tensor_reduce(
                    out=lg[:, :].rearrange("p (k m) -> p k m", m=L),
                    in_=prod[:, :].rearrange("p (k l m) -> p k m l", l=L, m=L),
                    axis=AX.X,
                    op=ALU.add,
                )
                # exp; |logits| < ~20 so fp32 exp is safe without max-sub
                st.e = small.tile([P, kp * L], FP32, tag=f"e{kp}",
                                  name=f"e_{uid}")
                nc.scalar.activation(st.e[:, :], lg[:, :], AF.Exp)
                se = small.tile([P, kp], FP32, tag=f"se{kp}", name=f"se_{uid}")
                nc.vector.tensor_reduce(
                    out=se[:, :],
                    in_=st.e[:, :].rearrange("p (k m) -> p k m", m=L),
                    axis=AX.X,
                    op=ALU.add,
                )
                st.recip = small.tile([P, kp], FP32, tag=f"rc{kp}",
                                      name=f"rc_{uid}")
                nc.vector.reciprocal(st.recip[:, :], se[:, :])

            def diag_build(st: Grp):
                """DVE: diag(e_j) tiles for every block of the group, in a
                single tensor_tensor op (ident and e broadcast against each
                other) — one instruction instead of kp*L tiny ones."""
                uid, kp = st.uid, st.kp
                dg = dpool.tile([P, kp * L * P], BF16, tag=f"dg{kp}",
                                name=f"dg_{uid}")
                st.dg = dg
                for k in range(kp):
                    for j in range(L):
                        nc.vector.tensor_scalar_mul(
                            dg[:, (k * L + j) * P : (k * L + j + 1) * P],
                            id_t[:, :],
                            st.e[:, k * L + j : k * L + j + 1],
                        )

            def mm_copy(st: Grp):
                """PE burst (kp*L diagonal matmuls) + normalize-copies."""
                uid, kp = st.uid, st.kp
                pool = opool if kp == 4 else otail
                tag = f"o{kp}"
                o_t = pool.tile([P, kp * D], BF16, tag=tag, name=f"o_{uid}")
                st.o_t = o_t
                dg = st.dg
                pss = []
                for k in range(kp):
                    ps = pspool.tile([P, D], FP32, tag="ps",
                                     name=f"ps_{uid}_{k}")
                    pss.append(ps)
                    for j in range(L):
                        nc.tensor.matmul(
                            ps[:, :],
                            dg[:, (k * L + j) * P : (k * L + j + 1) * P],
                            st.xt[:, k, j, :],
                            start=(j == 0),
                            stop=(j == L - 1),
                        )
                for k in range(kp):
                    if k < N_DVE_COPIES:
                        nc.vector.tensor_scalar_mul(
                            o_t[:, k * D : (k + 1) * D], pss[k][:, :],
                            st.recip[:, k : k + 1],
                        )
                    else:
                        nc.scalar.activation(
                            o_t[:, k * D : (k + 1) * D], pss[k][:, :], AF.Copy,
                            scale=st.recip[:, k : k + 1],
                        )
                    # keep-warm: a tiny matmul chained to each copy keeps the
                    # Tensor engine's busy streak alive through the idle
                    # window between group bursts, so the p-state model never
                    # demotes the next burst's clock.
                    dum = psdpool.tile([8, 8], FP32, tag="dum",
                                       name=f"dum_{uid}_{k}")
                    nc.tensor.matmul(
                        dum[:, :], id_t[:, 0:8],
                        o_t[:, k * D : k * D + 8],
                        start=True, stop=True,
                    )

            def store_group(st: Grp):
                nc.sync.dma_start(
                    out=out[st.b0 * P : (st.b0 + st.kp) * P, :].rearrange(
                        "(k p) d -> p k d", p=P
                    ),
                    in_=st.o_t[:, :].rearrange("p (k d) -> p k d", d=D),
                )

            # Software pipeline, 2-deep while loads remain:
            #   load(g+1) ; stage_a(g)+diags(g) ; matmuls+copies(g-1) ;
            #   store(g-2)
            # diags(g) are emitted at the end of stage_a(g), so when the
            # Tensor engine starts group g's burst (next iteration) every
            # diagonal is already resident — the burst never stalls and the
            # PE p-state ramps to full clock.
            NG = len(groups)
            hist: list[Grp] = []      # groups whose mm_copy is not yet emitted
            unstored: list[Grp] = []  # groups computed but not yet stored
            for rep in range(reps):
                for gi in range(NG):
                    uid = rep * NG + gi
                    st = Grp()
                    st.uid = uid
                    st.gi = gi
                    st.b0, st.kp = groups[gi]
                    st.xt = first_x if uid == 0 else cur_x  # noqa: F821
                    last = uid + 1 >= reps * NG
                    if not last:
                        cur_x = load_group((uid + 1) % NG, (uid + 1) // NG)
                        stage_a(st)
                        diag_build(st)
                        if hist:
                            done = hist.pop(0)
                            mm_copy(done)
                            unstored.append(done)
                        # store with a 2-group lag behind mm_copy so the
                        # trigger's o_t dependency is always satisfied
                        while len(unstored) > 3:
                            store_group(unstored.pop(0))
                        hist.append(st)
                    else:
                        # all loads issued: drain
                        stage_a(st)
                        diag_build(st)
                        while hist:
                            done = hist.pop(0)
                            mm_copy(done)
                            unstored.append(done)
                        while len(unstored) > 1:
                            store_group(unstored.pop(0))
                        mm_copy(st)
                        unstored.append(st)
                        while unstored:
                            store_group(unstored.pop(0))

    # Legalize for TRN2 (≤1 sync wait per instruction) + register alloc.
    nc.compile()
    return nc


_MODULE_CACHE: bass.Bass | None = None


def _get_module() -> bass.Bass:
    global _MODULE_CACHE
    if _MODULE_CACHE is None:
        _MODULE_CACHE = build_module()
    return _MODULE_CACHE


def make_in_maps(inputs: dict) -> list:
    bf16 = mybir.dt.np(BF16)
    # Ws[s, l, m] -> ws[p, (kb l m)] with s = kb*128 + p
    ws = (
        np.asarray(inputs["Ws"], dtype=np.float32)
        .reshape(4, P, L * L)
        .transpose(1, 0, 2)
        .reshape(P, 4 * L * L)
    ) * float(D)
    ws = np.ascontiguousarray(ws)
    ident = np.eye(P, dtype=bf16)
    # Per core, row r = bi*512 + s;  block b = r//128, p = r%128.
    # Device layout: xall[b*P + p, j*D + d] = x_j[bi, s, d]
    xs = np.stack(
        [np.asarray(inputs[f"x{j}"]).astype(bf16) for j in range(L)], axis=0
    )  # [J, B, S, D]
    in_maps = []
    for c in range(N_CORES):
        xc = xs[:, c * B_PER : (c + 1) * B_PER]          # [J, B_PER, S, D]
        xc = xc.reshape(L, NB, P, D).transpose(1, 2, 0, 3)
        m = {
            "xall": np.ascontiguousarray(xc).reshape(NB * P, JD),
            "ws": ws,
            "ident": ident,
        }
        in_maps.append(m)
    return in_maps


def kernel(**inputs) -> np.ndarray:
    nc = _get_module()
    in_maps = make_in_maps(inputs)
    res = run_bass_kernel_spmd(nc, in_maps, core_ids=list(range(N_CORES)))
    outs = []
    for c in range(N_CORES):
        oc = np.asarray(res.results[c]["out"])            # [NB*P, D] bf16
        oc = oc.reshape(NB, P, D).reshape(B_PER, S, D)
        outs.append(oc.astype(np.float32))
    return np.concatenate(outs, axis=0)
